# revision 1
# baseline (speedup 1.0000x reference)
"""Trainium2 Bass kernel for nn_DSRLossStateless (DSR loss, stateless).

loss = -sum_t(D_t)/B where D_t comes from an eta-EMA pair (A,B) over
portfolio returns R_t = sum_a w[t,a]*nr[t,a].

Strategy (8 cores, batch-sharded):
  - Each core owns 250k consecutive rows, laid out as SBUF partitions
    1..125 x 2000 columns (time-major within a partition). Partition 0
    holds the 2000 rows preceding the core's span (synthetic for core 0),
    which supplies the scan carry for partition 1.
  - Stage A (memory-bound bulk): tiled DMA of w/nr, elementwise product
    (split GPSIMD/DVE), segmented rowsum-of-16 on DVE -> R. Per chunk,
    ACT derives eta*R, eta*R^2, R^2 and DVE extends chained local scans
    (zero-carry) for A_loc/B_loc - all overlapped with the DMA stream.
  - Tail: per-partition carries are just the previous partition's local
    scan final (c^2000 ~ 1.9e-9 kills older terms), applied as
    A_prev = A_loc_shift + carry*c^t in one fused op per scan; then a
    short elementwise chain computes q_t = -D_t/eta and fused reduces
    leave one partial per core.
  - Host: loss = eta * sum(partials) / B.
"""

import sys

sys.path.insert(0, "/opt/trn_rl_repo")

import numpy as np

import concourse.bass as bass
import concourse.bacc as bacc
import concourse.tile as tile
from concourse import mybir
from concourse.bass_utils import run_bass_kernel_spmd
from contextlib import ExitStack

F32 = mybir.dt.float32
NF32 = np.float32

N_CORES = 8
NA = 16            # assets (inner dim)
KP = 126           # SBUF partitions used (0 = prepend/carry-feeder)
L = 2000           # columns (time steps per partition)
LE = L + 1         # local-scan buffer width (col 0 = zero carry)
OWN = (KP - 1) * L      # rows owned per core = 250000
B_TOTAL = N_CORES * OWN # 2000000
CH = 16            # stage-A chunks
KC = L // CH       # 125 rows per partition per chunk
FW = KC * NA       # 2000 f32 per partition per chunk tile
ETA = 0.01
EPS = 1e-8
CDEC = NF32(1.0 - ETA)  # 0.99

AL = mybir.AluOpType
AF = mybir.ActivationFunctionType
AX = mybir.AxisListType

_PROGRAM = None


def _build_program():
    nc = bacc.Bacc("TRN2", target_bir_lowering=False, debug=False)

    w_ap = nc.dram_tensor("w", [KP * L, NA], F32, kind="ExternalInput").ap()
    nr_ap = nc.dram_tensor("nr", [KP * L, NA], F32, kind="ExternalInput").ap()
    out_ap = nc.dram_tensor("out", [1, 1], F32, kind="ExternalOutput").ap()

    # geo_c[t] = c^t (carry decay for the correction pass)
    geoc_np = (CDEC ** np.arange(L).astype(NF32)).astype(NF32)
    geoc_dram = nc.inline_tensor(
        np.ascontiguousarray(np.broadcast_to(geoc_np, (KP, L))), name="geoc"
    )

    w_v = w_ap.rearrange("(p t) a -> p (t a)", p=KP)
    nr_v = nr_ap.rearrange("(p t) a -> p (t a)", p=KP)

    with tile.TileContext(nc) as tc, ExitStack() as ctx:
        pers = ctx.enter_context(tc.tile_pool(name="pers", bufs=1))
        loadp = ctx.enter_context(tc.tile_pool(name="load", bufs=4))
        tmpp = ctx.enter_context(tc.tile_pool(name="tmp", bufs=1))

        R = pers.tile([KP, L], F32, tag="R")
        R2 = pers.tile([KP, L], F32, tag="R2")
        etaR = pers.tile([KP, L], F32, tag="etaR")
        etaR2 = pers.tile([KP, L], F32, tag="etaR2")
        Aloc = pers.tile([KP, LE], F32, tag="Aloc")
        Bloc = pers.tile([KP, LE], F32, tag="Bloc")
        Aprev = pers.tile([KP, L], F32, tag="Aprev")
        Bprev = pers.tile([KP, L], F32, tag="Bprev")
        cvec = pers.tile([KP, KC], F32, tag="cvec")
        geoc = pers.tile([KP, L], F32, tag="geoc")
        initA = pers.tile([KP, 1], F32, tag="initA")
        initB = pers.tile([KP, 1], F32, tag="initB")
        qsum = pers.tile([KP, 1], F32, tag="qsum")
        qrow = pers.tile([1, KP - 1], F32, tag="qrow")
        qtot = pers.tile([1, 1], F32, tag="qtot")

        # constants / scan seeds
        nc.vector.memset(qtot[0:1, 0:1], 1.0)
        nc.scalar.sqrt(qtot[0:1, 0:1], qtot[0:1, 0:1])  # pin ACT table early
        nc.vector.memset(cvec[:, :], float(CDEC))
        nc.vector.memset(Aloc[:, 0:1], 0.0)
        nc.vector.memset(Bloc[:, 0:1], 0.0)
        nc.vector.memset(initA[0:1, 0:1], 0.0)
        nc.vector.memset(initB[0:1, 0:1], 0.0)
        nc.sync.dma_start(geoc[:], geoc_dram.ap())

        # ---- stage A: chunked bulk + scan extension ----
        for k in range(CH):
            ks = slice(k * KC, (k + 1) * KC)
            wt = loadp.tile([KP, FW], F32, tag="wt")
            rt = loadp.tile([KP, FW], F32, tag="rt")
            nc.sync.dma_start(wt[:], w_v[:, k * FW:(k + 1) * FW])
            nc.scalar.dma_start(rt[:], nr_v[:, k * FW:(k + 1) * FW])
            eng = nc.vector if k == CH - 1 else nc.gpsimd
            eng.tensor_mul(wt[:], wt[:], rt[:])
            nc.vector.reduce_sum(
                R[:, ks], wt[:].rearrange("p (t a) -> p t a", a=NA), axis=AX.X
            )
            # derived streams on ACT
            nc.scalar.mul(etaR[:, ks], R[:, ks], ETA)
            nc.scalar.activation(etaR2[:, ks], R[:, ks], AF.Square, scale=0.1)
            nc.scalar.square(R2[:, ks], R[:, ks])
            # chained zero-carry local scans: state = c*state + eta*x
            nc.vector.tensor_tensor_scan(
                out=Aloc[:, 1 + k * KC:1 + (k + 1) * KC], data0=cvec[:, :],
                data1=etaR[:, ks], initial=Aloc[:, k * KC:k * KC + 1],
                op0=AL.mult, op1=AL.add,
            )
            nc.vector.tensor_tensor_scan(
                out=Bloc[:, 1 + k * KC:1 + (k + 1) * KC], data0=cvec[:, :],
                data1=etaR2[:, ks], initial=Bloc[:, k * KC:k * KC + 1],
                op0=AL.mult, op1=AL.add,
            )

        # ---- tail ----
        # carries: previous partition's local final
        nc.sync.dma_start(initA[1:KP, 0:1], Aloc[0:KP - 1, L:LE])
        nc.scalar.dma_start(initB[1:KP, 0:1], Bloc[0:KP - 1, L:LE])

        # A_prev[:,t] = Aloc[:,t-1] + initA*c^t  (Aloc col0 is the zero pad)
        nc.vector.scalar_tensor_tensor(
            out=Aprev[:, :], in0=geoc[:, :], scalar=initA[:, 0:1],
            in1=Aloc[:, 0:L], op0=AL.mult, op1=AL.add,
        )
        nc.vector.scalar_tensor_tensor(
            out=Bprev[:, :], in0=geoc[:, :], scalar=initB[:, 0:1],
            in1=Bloc[:, 0:L], op0=AL.mult, op1=AL.add,
        )

        # ---- D chain: q = [0.5*A*(R^2+B) - B*R] / var^1.5 ----
        T1 = tmpp.tile([KP, L], F32, tag="T1")
        T2 = tmpp.tile([KP, L], F32, tag="T2")
        T3 = tmpp.tile([KP, L], F32, tag="T3")
        T4 = tmpp.tile([KP, L], F32, tag="T4")
        T5 = tmpp.tile([KP, L], F32, tag="T5")
        T6 = tmpp.tile([KP, L], F32, tag="T6")

        nc.gpsimd.tensor_add(T1[:, :], R2[:, :], Bprev[:, :])      # g1 = R^2+B
        nc.gpsimd.tensor_mul(T2[:, :], Bprev[:, :], R[:, :])       # g3 = B*R
        nc.vector.scalar_tensor_tensor(                            # g2 = 0.5A*g1
            out=T3[:, :], in0=Aprev[:, :], scalar=0.5, in1=T1[:, :],
            op0=AL.mult, op1=AL.mult,
        )
        nc.vector.tensor_sub(T3[:, :], T3[:, :], T2[:, :])         # negn = g2-g3
        nc.scalar.square(T4[:, :], Aprev[:, :])                    # a2 = A^2
        nc.vector.tensor_sub(T5[:, :], Bprev[:, :], T4[:, :])      # v = B-a2
        nc.vector.tensor_scalar_max(T5[:, :], T5[:, :], EPS)       # var
        nc.scalar.sqrt(T6[:, :], T5[:, :])                         # s = sqrt(var)
        nc.vector.tensor_mul(T4[:, :], T5[:, :], T6[:, :])         # d = var^1.5
        nc.vector.reciprocal_approx_accurate(T5[:, :], T4[:, :], T6[:, :])  # rec

        nc.vector.scalar_tensor_tensor(                            # qsum=sum(negn*rec)
            out=T4[:, :], in0=T3[:, :], scalar=1.0, in1=T5[:, :],
            op0=AL.mult, op1=AL.mult, accum_out=qsum[:, 0:1],
        )
        # partition reduce: flatten 125 partials to one row, reduce, store
        nc.sync.dma_start(qrow[0:1, 0:KP - 1], qsum[1:KP, 0:1])
        nc.vector.reduce_sum(qtot[0:1, 0:1], qrow[0:1, 0:KP - 1], axis=AX.X)
        nc.sync.dma_start(out_ap[0:1, 0:1], qtot[0:1, 0:1])

    nc.compile()
    return nc


def _get_program():
    global _PROGRAM
    if _PROGRAM is None:
        _PROGRAM = _build_program()
    return _PROGRAM


def _core0_prepend():
    """2000 synthetic rows encoding the global init (A,B)=(0,EPS).

    All-zero rows leave the scan at (0,0); the last two rows carry returns
    r1, r2 with r2 = -fl(c*r1) so the A-scan cancels to ~0, while
    eta*(c*r1^2 + r2^2) ~ EPS supplies the B carry.
    """
    w = np.zeros((L, NA), NF32)
    nr = np.zeros((L, NA), NF32)
    c = CDEC
    r1 = NF32(np.sqrt(EPS / (ETA * (float(c) + float(c) ** 2))))
    r2 = NF32(-(c * r1))
    w[L - 2, 0] = NF32(1.0)
    nr[L - 2, 0] = r1
    w[L - 1, 0] = NF32(1.0)
    nr[L - 1, 0] = r2
    return w, nr


def _make_in_maps(weights, nr):
    weights = np.ascontiguousarray(weights, dtype=NF32)
    nr = np.ascontiguousarray(nr, dtype=NF32)
    pre_w, pre_nr = _core0_prepend()
    in_maps = []
    for m in range(N_CORES):
        s = m * OWN
        if m == 0:
            wm = np.concatenate([pre_w, weights[:OWN]])
            rm = np.concatenate([pre_nr, nr[:OWN]])
        else:
            wm = weights[s - L:s + OWN]
            rm = nr[s - L:s + OWN]
        in_maps.append({"w": wm, "nr": rm})
    return in_maps


def _run(in_maps, **kwargs):
    nc = _get_program()
    return run_bass_kernel_spmd(nc, in_maps, core_ids=list(range(N_CORES)), **kwargs)


def kernel(weights, next_returns):
    in_maps = _make_in_maps(weights, next_returns)
    res = _run(in_maps)
    total = np.sum(
        np.array([res.results[m]["out"][0, 0] for m in range(N_CORES)], NF32),
        dtype=NF32,
    )
    return NF32(NF32(ETA) * total / NF32(B_TOTAL))

